# revision 21
# baseline (speedup 1.0000x reference)
"""Trainium2 Bass kernel for the 3-layer weighted GraphConv encoder (v2).

Strategy (8 NeuronCores, SPMD single NEFF):
- All three layers are edge-sharded by DST node range: each core owns a
  contiguous slice of output rows (6250 movies for layer 1, 2500 users for
  layers 2+3), so the segment-sums need no collective.
- Layers 2 and 3 share the same edge list, so their messages are fetched with
  ONE dma_gather per edge from a merged [N_M, 2D] bf16 table whose rows are
  [x_meas | movie_x]. The movie_x half is produced by layer 1 and distributed
  with a chunked AllGather that overlaps the layer-1 linear.
- Everything on the PE path is bf16 (4x faster matmul than fp32); PSUM
  accumulates in fp32; gathers move half the bytes.
- Segment sums on the tensor engine: per 128-edge chunk a
  S[e, s] = (dstloc[e] == s) * sigmoid(ew[e]) selection matrix is built with
  one dual-op tensor_scalar (bf16), and psum[feat, seg] += msg^T @ S.
- dma_gather calls rotate across the 4 SWDGE queues; each queue runs on its
  own Q7 core pair, so up to 4 descriptor generations proceed concurrently.

The per-window chunk schedule is data-dependent; it is computed from the
actual inputs at kernel() time (max over cores per window) and baked into the
program, with per-core padding to the shared schedule (padded edges have
dstloc = -1 so their S row is all zeros).
"""

import math

import numpy as np
import orjson

import concourse.bass as bass
import concourse.mybir as mybir
import concourse.tile as tile
from concourse import library_config
from concourse.library_overlay import lower_extended_insts
from concourse.bass_utils import run_bass_kernel_spmd

try:
    import ml_dtypes
    BF16_NP = ml_dtypes.bfloat16
except Exception:  # pragma: no cover
    BF16_NP = None

# ---------------------------------------------------------------- BIR legalize
_DMA_OPCODES = {
    "DMACopy", "TensorLoad", "TensorSave", "ISA", "CollectiveCompute",
    "DmaTranspose", "TriggerDma",
}
_lg_counter = [0]


def _lg_nop(inst, wait=None, update=None):
    _lg_counter[0] += 1
    return {
        "name": f"lg{_lg_counter[0]}",
        "opcode": "EventSemaphore",
        "engine": inst["engine"],
        "ins": [],
        "outs": [],
        "debug": inst.get("debug", 0),
        "sync_info": {
            "on_wait": [wait] if wait else [],
            "on_update": [update] if update else [],
        },
    }


def _lg_walk(block, stats):
    out = []
    for inst in block.get("instructions", []):
        si = inst.get("sync_info")
        trailing = []
        if si:
            ows = si.get("on_wait") or []
            if len(ows) > 1:
                stats[0] += len(ows) - 1
                for w in ows[:-1]:
                    out.append(_lg_nop(inst, wait=w))
                si["on_wait"] = [ows[-1]]
            ous = si.get("on_update") or []
            if len(ous) > 1 and inst.get("opcode") not in _DMA_OPCODES:
                stats[1] += len(ous) - 1
                for u in ous[1:]:
                    trailing.append(_lg_nop(inst, update=u))
                si["on_update"] = [ous[0]]
        out.append(inst)
        out.extend(trailing)
    block["instructions"] = out
    for sb in block.get("blocks") or []:
        _lg_walk(sb, stats)


def legalize_bir_json(bir_json: bytes) -> bytes:
    d = orjson.loads(bir_json)
    stats = [0, 0]
    for fn in d.get("functions", []):
        for b in fn.get("blocks", []):
            _lg_walk(b, stats)
    return orjson.dumps(d)


def _install_birfix():
    import concourse.bass_utils as bu
    import concourse.bass2jax as b2j

    if getattr(bu, "_birfix_installed", False):
        return
    orig = bu.compile_bir_kernel

    def wrapper(bir_json, tmpdir, neff_name="file.neff"):
        return orig(legalize_bir_json(bir_json), tmpdir, neff_name)

    bu.compile_bir_kernel = wrapper
    bu._birfix_installed = True
    b2j.compile_bir_kernel = wrapper


# ------------------------------------------------------------------- constants
N_M, N_D, E, D, H, O = 50000, 20000, 600000, 128, 128, 64
NC = 8
P = 128
RM = N_M // NC            # 6250 movie rows per core
RU = N_D // NC            # 2500 user rows per core
W1 = math.ceil(RM / P)    # 49 windows for layer 1
W2B = math.ceil(RU / P)   # 20 windows for layers 2+3
SPLIT = 32768             # int16-addressable row limit
NB = 8                    # chunks per dma_gather batch (1024 idx/call)
NB23 = 8                  # chunks per batch for the l23 streams
NAG = 5                   # AllGather chunks
F32 = mybir.dt.float32
BF16 = mybir.dt.bfloat16
I16 = mybir.dt.int16
DEBUG = False
GATHER_ONLY = False


# ---------------------------------------------------------------- host-side prep
def _pack_schedule(counts):
    mx = counts.max(axis=0)
    return (mx + P - 1) // P


def _build_streams(nwin, core, win, grp, ngrp, src_local, dstloc, ew):
    """Partition/pad edges into per-(window, group) runs shared across cores."""
    counts = np.zeros((NC, nwin, ngrp), np.int64)
    np.add.at(counts, (core, win, grp), 1)
    sched = _pack_schedule(counts)                      # [nwin, ngrp]
    run_len = sched * P
    off = np.zeros((nwin, ngrp), np.int64)
    for g in range(ngrp):
        off[:, g] = np.concatenate(([0], np.cumsum(run_len[:, g])[:-1]))
    stream_len = run_len.sum(axis=0)

    # sort each (core, grp, win) run's edges by ascending src so the
    # gather's HBM reads are semi-sequential (better DRAM row locality)
    order = np.lexsort((src_local, win, grp, core))
    inv = np.empty_like(order)
    inv[order] = np.arange(len(order))
    flat = ((core * nwin) + win) * ngrp + grp
    sort_flat = flat[order]
    starts = np.concatenate(([0], np.nonzero(np.diff(sort_flat))[0] + 1))
    run_start_per_pos = np.zeros(len(order), np.int64)
    run_start_per_pos[starts] = starts
    run_start_per_pos = np.maximum.accumulate(run_start_per_pos)
    rank_sorted = np.arange(len(order)) - run_start_per_pos
    rank = rank_sorted[inv]

    pos = off[win, grp] + rank
    streams = []
    for c in range(NC):
        per = {}
        for g in range(ngrp):
            n = int(stream_len[g])
            idx_a = np.zeros(n, np.int16)
            dst_a = np.full(n, -1.0, np.float32)
            ew_a = np.zeros(n, np.float32)
            m = (core == c) & (grp == g)
            idx_a[pos[m]] = src_local[m].astype(np.int16)
            dst_a[pos[m]] = dstloc[m].astype(np.float32)
            if ew is not None:
                ew_a[pos[m]] = ew[m]
            per[g] = (idx_a, dst_a, ew_a)
        streams.append(per)
    return sched, streams


def _pack_idx_dma(idx_a, NBs=NB):
    """idx stream [nchunks*128] int16 -> DMA layout [128, ncalls*NBs*8]."""
    nchunks = len(idx_a) // P
    ncalls = max(math.ceil(nchunks / NBs), 1)
    out = np.zeros((P, ncalls * NBs * 8), np.int16)
    for k in range(ncalls):
        nb = min(NBs, nchunks - k * NBs)
        call = idx_a[k * NBs * P: k * NBs * P + nb * P]
        blk = call.reshape(nb * 8, 16).T               # [16, nb*8]
        out[:, k * NBs * 8: k * NBs * 8 + nb * 8] = np.tile(blk, (8, 1))
    return out


def _pack_col_major(a):
    return np.ascontiguousarray(a.reshape(-1, P).T)


def _prep_layer1(src_m, dst_m):
    core = dst_m // RM
    dloc = dst_m % RM
    win = dloc // P
    dstloc = dloc % P
    grp = (src_m >= SPLIT).astype(np.int64)
    src_local = np.where(grp == 0, src_m, src_m - SPLIT)
    return _build_streams(W1, core, win, grp, 2, src_local, dstloc, None)


def _prep_l23(src_b, dst_b, ew):
    core = dst_b // RU
    dloc = dst_b % RU
    win = dloc // P
    dstloc = dloc % P
    grp = (src_b >= SPLIT).astype(np.int64)
    src_local = np.where(grp == 0, src_b, src_b - SPLIT)
    return _build_streams(W2B, core, win, grp, 2, src_local, dstloc, ew)


# --------------------------------------------------------------- device program
class _Stream:
    """On-device cursor over one gather stream. All idx/dst/ew data is
    preloaded to SBUF; batches of nb chunks are gathered on demand;
    next() yields (msg_ap, dst_col, ew_col)."""

    def __init__(self, nc, sb, name, nchunks, idx_t, dst_t, ews_t,
                 table_ap, reg_cache, width=D, nb=NB):
        self.nc, self.sb, self.name = nc, sb, name
        self.reg_cache = reg_cache
        self.width = width
        self.nchunks = nchunks
        self.idx_t, self.dst_t, self.ews_t = idx_t, dst_t, ews_t
        self.table_ap = table_ap
        self.nb = nb
        self.pos = 0
        self.msg = None

    def _fetch(self, k):
        nc, sb = self.nc, self.sb
        NBs = self.nb
        nb = min(NBs, self.nchunks - k * NBs)
        mtag = "msg1" if self.width == D else "msg2"
        mbufs = 12 if self.width == D else 18
        self.msg = sb.tile([P, NBs, self.width], BF16, tag=mtag, name="msgt",
                           bufs=mbufs)
        v = nb * P
        if v not in self.reg_cache:
            self.reg_cache[v] = nc.gpsimd.to_reg(v)
        q = self.reg_cache.setdefault("_q", [0])
        nc.gpsimd.dma_gather(self.msg[:, :nb, :], self.table_ap,
                             self.idx_t[:, k * NBs * 8: k * NBs * 8 + nb * 8],
                             v, self.reg_cache[v], self.width,
                             queue_num=q[0] % 4)
        q[0] += 1

    def next(self):
        k, slot = divmod(self.pos, self.nb)
        if slot == 0:
            self._fetch(k)
        c = self.pos
        self.pos += 1
        ew_col = (self.ews_t[:, c:c + 1] if self.ews_t is not None else None)
        return self.msg[:, slot, :], self.dst_t[:, c:c + 1], ew_col


def _build_program(sched1, sched23):
    nc = bass.Bass(trn_type="TRN2", num_devices=NC, num_swdge_queues=4)

    # ---- kernel I/O ----
    xm = nc.dram_tensor("xm", [N_M, D], BF16, kind="ExternalInput")
    t23 = nc.dram_tensor("t23", [N_M, 2 * D], BF16, kind="ExternalInput")
    xmT = nc.dram_tensor("xmT", [P, W1 * P], BF16, kind="ExternalInput")
    xdT = nc.dram_tensor("xdT", [P, W2B * P], BF16, kind="ExternalInput")
    wts = {}
    for nm, shape in [("W_rel1", [D, H]), ("W_root1", [D, H]),
                      ("W_rel2", [D, H]), ("W_root2", [D, H]),
                      ("W_rel3", [H, H]), ("W_root3", [H, H]),
                      ("W_lin", [H, O])]:
        wts[nm] = nc.dram_tensor(nm, shape, BF16, kind="ExternalInput")
    b1row = nc.dram_tensor("b1row", [1, H], BF16, kind="ExternalInput")
    b2col = nc.dram_tensor("b2col", [H, 1], F32, kind="ExternalInput")
    b3col = nc.dram_tensor("b3col", [H, 1], F32, kind="ExternalInput")
    blcol = nc.dram_tensor("blcol", [O, 1], F32, kind="ExternalInput")
    iota_in = nc.dram_tensor("iota", [P, P], BF16, kind="ExternalInput")
    ones1 = nc.dram_tensor("ones1", [1, P], BF16, kind="ExternalInput")
    out = nc.dram_tensor("out", [O, RU], F32, kind="ExternalOutput")
    if DEBUG:
        dbg = {
            "dbg_agg1": nc.dram_tensor("dbg_agg1", [P, W1 * P], BF16,
                                       kind="ExternalOutput"),
            "dbg_agg2": nc.dram_tensor("dbg_agg2", [P, W2B * P], BF16,
                                       kind="ExternalOutput"),
            "dbg_agg3": nc.dram_tensor("dbg_agg3", [P, W2B * P], BF16,
                                       kind="ExternalOutput"),
            "dbg_user2": nc.dram_tensor("dbg_user2", [P, W2B * P], BF16,
                                        kind="ExternalOutput"),
            "dbg_mslice": nc.dram_tensor("dbg_mslice", [RM, H], BF16,
                                         kind="ExternalOutput"),
            "dbg_table": nc.dram_tensor("dbg_table", [4000, 2 * D], BF16,
                                        kind="ExternalOutput"),
        }

    def idx_cols(n, NBs=NB):
        return max(math.ceil(n / NBs) * NBs * 8, NBs * 8)

    sdefs = [("l1lo", int(sched1[:, 0].sum()), False),
             ("l1hi", int(sched1[:, 1].sum()), False),
             ("l23lo", int(sched23[:, 0].sum()), True),
             ("l23hi", int(sched23[:, 1].sum()), True)]
    sdram = {}
    for nm, nch, has_ew in sdefs:
        nbs = NB23 if nm.startswith("l23") else NB
        sdram[nm] = (
            nc.dram_tensor(f"{nm}_idx", [P, idx_cols(nch, nbs)], I16,
                           kind="ExternalInput"),
            nc.dram_tensor(f"{nm}_dst", [P, max(nch, 1)], F32,
                           kind="ExternalInput"),
            nc.dram_tensor(f"{nm}_ew", [P, max(nch, 1)], F32,
                           kind="ExternalInput") if has_ew else None,
        )

    Relu = mybir.ActivationFunctionType.Relu
    Ident = mybir.ActivationFunctionType.Identity

    # AllGather chunk boundaries in layer-1 windows / movie rows
    wsplit = [0, 10, 20, 30, 40, W1]
    rsplit = [min(w * P, RM) for w in wsplit]

    with tile.TileContext(nc) as tc:
        with (
            tc.tile_pool(name="sbuf", bufs=4) as sb,
            tc.tile_pool(name="big", bufs=1) as bigp,
            tc.tile_pool(name="psum", bufs=4, space="PSUM") as ps,
            tc.tile_pool(name="psl", bufs=2, space="PSUM") as psl,
            tc.tile_pool(name="dram", bufs=1, space="DRAM") as dr,
        ):
            nc.gpsimd.load_library(library_config.mlp)

            # constants / weights to SBUF
            iota_f = bigp.tile([P, P], BF16, tag="iota")
            nc.scalar.dma_start(iota_f[:], iota_in[:])
            ones1_t = bigp.tile([1, P], BF16, tag="ones1")
            nc.scalar.dma_start(ones1_t[:], ones1[:])
            wt = {}
            for nm in wts:
                shp = [D, H] if nm != "W_lin" else [H, O]
                wt[nm] = bigp.tile(shp, BF16, tag=nm, name=nm + "_t")
                nc.scalar.dma_start(wt[nm][:], wts[nm][:])
            b1row_t = bigp.tile([1, H], BF16, tag="b1row")
            nc.scalar.dma_start(b1row_t[:], b1row[:])
            bcol_t = {}
            for nm, t in [("b2", b2col), ("b3", b3col), ("bl", blcol)]:
                bcol_t[nm] = bigp.tile([t.shape[0], 1], F32, tag="bc_" + nm,
                                       name="bc_" + nm)
                nc.scalar.dma_start(bcol_t[nm][:], t[:])

            xmT_t = bigp.tile([P, W1 * P], BF16, tag="xmT")
            nc.scalar.dma_start(xmT_t[:], xmT[:])
            xdT_t = bigp.tile([P, W2B * P], BF16, tag="xdT")
            nc.scalar.dma_start(xdT_t[:], xdT[:])

            agg1 = bigp.tile([P, W1 * P], BF16, tag="agg1")
            agg2 = bigp.tile([P, W2B * P], BF16, tag="agg2")
            agg3 = bigp.tile([P, W2B * P], BF16, tag="agg3")
            user2 = bigp.tile([P, W2B * P], BF16, tag="user2")
            user3 = bigp.tile([P, W2B * P], BF16, tag="user3")
            outT = bigp.tile([O, W2B * P], F32, tag="outT")

            mslice = dr.tile([RM, H], BF16, tag="mslice")
            maggs = [dr.tile([NC, rsplit[k + 1] - rsplit[k], H], BF16,
                             tag=f"magg{k}", name=f"magg{k}")
                     for k in range(NAG)]

            reg_cache = {}
            nchd = {s[0]: s[1] for s in sdefs}
            Sig = mybir.ActivationFunctionType.Sigmoid
            pre = {}
            for nm, nch, has_ew in sdefs:
                idx_d, dst_d, ew_d = sdram[nm]
                it = bigp.tile(list(idx_d.shape), I16, tag=f"pidx_{nm}",
                               name=f"pidx_{nm}")
                nc.sync.dma_start(it[:], idx_d[:])
                dt_ = bigp.tile(list(dst_d.shape), F32, tag=f"pdst_{nm}",
                                name=f"pdst_{nm}")
                nc.sync.dma_start(dt_[:], dst_d[:])
                et = None
                if ew_d is not None:
                    ewr = bigp.tile(list(ew_d.shape), F32, tag=f"pewr_{nm}",
                                    name=f"pewr_{nm}")
                    nc.sync.dma_start(ewr[:], ew_d[:])
                    et = bigp.tile(list(ew_d.shape), F32, tag=f"pews_{nm}",
                                   name=f"pews_{nm}")
                    nc.scalar.activation(et[:], ewr[:], Sig)
                pre[nm] = (it, dt_, et)

            def mkstream(nm, table_ap, width, nb=NB):
                it, dt_, et = pre[nm]
                return _Stream(nc, sb, nm, nchd[nm], it[:], dt_[:],
                               et[:] if et is not None else None,
                               table_ap, reg_cache, width=width, nb=nb)

            # ---------------- layer 1: segment sum + linear ----------------
            s1 = [mkstream("l1lo", xm[:], D), mkstream("l1hi", xm[SPLIT:, :], D)]
            agk = 0
            for w in range(W1):
                ntot = int(sched1[w].sum())
                dstc = agg1[:, w * P:(w + 1) * P]
                if ntot == 0 or GATHER_ONLY:
                    nc.vector.memset(dstc, 0.0)
                    for g in range(2):
                        for _ in range(int(sched1[w][g])):
                            s1[g].next()
                else:
                    acc = ps.tile([P, P], F32, tag="win")
                    j = 0
                    for g in range(2):
                        for _ in range(int(sched1[w][g])):
                            msg_ap, dcol, _ = s1[g].next()
                            S = sb.tile([P, P], BF16, tag="S", bufs=48)
                            nc.vector.tensor_scalar(
                                out=S[:], in0=iota_f[:], scalar1=dcol,
                                scalar2=None, op0=mybir.AluOpType.is_equal)
                            nc.tensor.matmul(out=acc[:], lhsT=msg_ap, rhs=S[:],
                                             start=(j == 0), stop=(j == ntot - 1))
                            j += 1
                    nc.scalar.activation(dstc, acc[:], Ident)

                # layer-1 linear for this window -> movie slice rows
                rows = min(P, RM - w * P)
                pt = psl.tile([P, H], F32, tag="lin")
                sl = slice(w * P, w * P + P)
                nc.tensor.matmul(out=pt[:], lhsT=ones1_t[:1, :],
                                 rhs=b1row_t[:1, :], start=True, stop=False)
                nc.tensor.matmul(out=pt[:], lhsT=agg1[:, sl],
                                 rhs=wt["W_rel1"][:], start=False, stop=False)
                nc.tensor.matmul(out=pt[:], lhsT=xmT_t[:, sl],
                                 rhs=wt["W_root1"][:], start=False, stop=True)
                mt = sb.tile([P, H], BF16, tag="mv", bufs=4)
                nc.scalar.activation(mt[:rows, :], pt[:rows, :], Relu)
                nc.scalar.dma_start(mslice[w * P: w * P + rows, :], mt[:rows, :])

                # fire AllGather chunk when its windows are done
                if w == wsplit[agk + 1] - 1:
                    a, b = rsplit[agk], rsplit[agk + 1]
                    nc.gpsimd.collective_compute(
                        "AllGather", mybir.AluOpType.bypass,
                        replica_groups=[list(range(NC))],
                        ins=[mslice[a:b, :].opt()],
                        outs=[maggs[agk][:].opt()])
                    for r in range(NC):
                        nc.scalar.dma_start(
                            t23[r * RM + a: r * RM + b, D:2 * D],
                            maggs[agk][r, :, :])
                    agk += 1

            # ---------------- layers 2+3: merged segment sum ----------------
            s23 = [mkstream("l23lo", t23[:], 2 * D, nb=NB23),
                   mkstream("l23hi", t23[SPLIT:, :], 2 * D, nb=NB23)]
            for w in range(W2B):
                ntot = int(sched23[w].sum())
                d2 = agg2[:, w * P:(w + 1) * P]
                d3 = agg3[:, w * P:(w + 1) * P]
                if ntot == 0 or GATHER_ONLY:
                    nc.vector.memset(d2, 0.0)
                    nc.vector.memset(d3, 0.0)
                    for g in range(2):
                        for _ in range(int(sched23[w][g])):
                            s23[g].next()
                    continue
                acc2 = ps.tile([P, P], F32, tag="win", name="acc2")
                acc3 = ps.tile([P, P], F32, tag="win", name="acc3")
                j = 0
                for g in range(2):
                    for _ in range(int(sched23[w][g])):
                        msg_ap, dcol, ecol = s23[g].next()
                        S = sb.tile([P, P], BF16, tag="S", bufs=48)
                        nc.vector.tensor_scalar(
                            out=S[:], in0=iota_f[:], scalar1=dcol, scalar2=ecol,
                            op0=mybir.AluOpType.is_equal,
                            op1=mybir.AluOpType.mult)
                        nc.tensor.matmul(out=acc2[:], lhsT=msg_ap[:, 0:D],
                                         rhs=S[:], start=(j == 0),
                                         stop=(j == ntot - 1))
                        nc.tensor.matmul(out=acc3[:], lhsT=msg_ap[:, D:2 * D],
                                         rhs=S[:], start=(j == 0),
                                         stop=(j == ntot - 1))
                        j += 1
                nc.scalar.activation(d2, acc2[:], Ident)
                nc.scalar.activation(d3, acc3[:], Ident)

            # ---------------- final linears (feature-major) -----------------
            SW = W2B * P
            for t in range(SW // 512):
                sl = slice(t * 512, (t + 1) * 512)
                pt = psl.tile([P, 512], F32, tag="lin2", name="lin2")
                nc.tensor.matmul(out=pt[:], lhsT=wt["W_rel2"][:],
                                 rhs=agg2[:, sl], start=True, stop=False)
                nc.tensor.matmul(out=pt[:], lhsT=wt["W_root2"][:],
                                 rhs=xdT_t[:, sl], start=False, stop=True)
                nc.scalar.activation(user2[:, sl], pt[:], Relu,
                                     bias=bcol_t["b2"][:])
            for t in range(SW // 512):
                sl = slice(t * 512, (t + 1) * 512)
                pt = psl.tile([P, 512], F32, tag="lin2", name="lin2")
                nc.tensor.matmul(out=pt[:], lhsT=wt["W_rel3"][:],
                                 rhs=agg3[:, sl], start=True, stop=False)
                nc.tensor.matmul(out=pt[:], lhsT=wt["W_root3"][:],
                                 rhs=user2[:, sl], start=False, stop=True)
                nc.scalar.activation(user3[:, sl], pt[:], Relu,
                                     bias=bcol_t["b3"][:])
            for t in range(SW // 512):
                sl = slice(t * 512, (t + 1) * 512)
                pt = psl.tile([O, 512], F32, tag="lin2", name="lin2")
                nc.tensor.matmul(out=pt[:], lhsT=wt["W_lin"][:],
                                 rhs=user3[:, sl], start=True, stop=True)
                nc.scalar.activation(outT[:, sl], pt[:], Ident,
                                     bias=bcol_t["bl"][:])

            nc.sync.dma_start(out[:], outT[:, :RU])
            if DEBUG:
                nc.sync.dma_start(dbg["dbg_agg1"][:], agg1[:])
                nc.sync.dma_start(dbg["dbg_agg2"][:], agg2[:])
                nc.sync.dma_start(dbg["dbg_agg3"][:], agg3[:])
                nc.sync.dma_start(dbg["dbg_user2"][:], user2[:])
                nc.sync.dma_start(dbg["dbg_mslice"][:], mslice[:])
                nc.sync.dma_start(dbg["dbg_table"][:], t23[0:4000, :])

    lower_extended_insts(nc)
    return nc


# ----------------------------------------------------------------------- kernel
def _bf(x):
    return np.asarray(x, np.float32).astype(BF16_NP)


def prepare(x_meas, x_dem, src_m, dst_m, src_b, dst_b, edge_weight,
            W_rel1, b_rel1, W_root1, W_rel2, b_rel2, W_root2,
            W_rel3, b_rel3, W_root3, W_lin, b_lin):
    _install_birfix()

    x_meas = np.asarray(x_meas, np.float32)
    x_dem = np.asarray(x_dem, np.float32)
    src_m = np.asarray(src_m, np.int64)
    dst_m = np.asarray(dst_m, np.int64)
    src_b = np.asarray(src_b, np.int64)
    dst_b = np.asarray(dst_b, np.int64)
    ew = np.asarray(edge_weight, np.float32)

    sched1, st1 = _prep_layer1(src_m, dst_m)
    sched23, st23 = _prep_l23(src_b, dst_b, ew)

    nc_prog = _build_program(sched1, sched23)

    iota = np.tile(np.arange(P, dtype=np.float32), (P, 1))
    ones1 = np.ones((1, P), np.float32)

    xm_bf = _bf(x_meas)
    t23_full = np.zeros((N_M, 2 * D), BF16_NP)
    t23_full[:, :D] = xm_bf

    def padT(x, cols):
        o = np.zeros((P, cols), BF16_NP)
        o[:, :x.shape[0]] = _bf(x).T
        return o

    in_maps = []
    for c in range(NC):
        m = {
            "xm": xm_bf,
            "t23": t23_full,
            "xmT": padT(x_meas[c * RM:(c + 1) * RM], W1 * P),
            "xdT": padT(x_dem[c * RU:(c + 1) * RU], W2B * P),
            "W_rel1": _bf(W_rel1), "W_root1": _bf(W_root1),
            "W_rel2": _bf(W_rel2), "W_root2": _bf(W_root2),
            "W_rel3": _bf(W_rel3), "W_root3": _bf(W_root3),
            "W_lin": _bf(W_lin),
            "b1row": _bf(np.asarray(b_rel1, np.float32).reshape(1, H)),
            "b2col": np.asarray(b_rel2, np.float32).reshape(H, 1),
            "b3col": np.asarray(b_rel3, np.float32).reshape(H, 1),
            "blcol": np.asarray(b_lin, np.float32).reshape(O, 1),
            "iota": iota.astype(BF16_NP),
            "ones1": ones1.astype(BF16_NP),
        }
        for nm, per in [("l1lo", st1[c][0]), ("l1hi", st1[c][1]),
                        ("l23lo", st23[c][0]), ("l23hi", st23[c][1])]:
            idx_a, dst_a, ew_a = per
            nbs = NB23 if nm.startswith("l23") else NB
            nchk = len(idx_a) // P
            m[f"{nm}_idx"] = (_pack_idx_dma(idx_a, nbs) if nchk
                              else np.zeros((P, nbs * 8), np.int16))
            m[f"{nm}_dst"] = (_pack_col_major(dst_a) if nchk
                              else np.zeros((P, 1), np.float32))
            if nm.startswith("l23"):
                m[f"{nm}_ew"] = (_pack_col_major(ew_a) if nchk
                                 else np.zeros((P, 1), np.float32))
        in_maps.append(m)

    return nc_prog, in_maps


def kernel(**inputs):
    nc_prog, in_maps = prepare(**inputs)
    res = run_bass_kernel_spmd(nc_prog, in_maps, core_ids=list(range(NC)))
    outs = [np.asarray(res.results[c]["out"], np.float32) for c in range(NC)]
    full = np.concatenate(outs, axis=1).T                 # [N_D, O]
    return np.ascontiguousarray(full, dtype=np.float32)
